# revision 35
# baseline (speedup 1.0000x reference)
"""Kohonen SOM distance-matrix kernel for Trainium2 (Bass/Tile).

Computes sqrt(||x||^2 + ||w||^2 - 2 x.w) for x [32768, 256] against a codebook
w [2500, 256] -> out [32768, 2500], data-parallel over 8 NeuronCores (batch
sharded, codebook replicated).

Per core (batch shard of 4096 rows, m-tiles of 128):
  - TensorE: fp8(e4m3) DoubleRow matmul (K=256 in one pass) computes
    cross = -2 x.w into PSUM; a second full-rank K=128 f16 matmul per slice
    accumulates BOTH norms on top: rows 0-63 carry wsq/64 x ones (column
    term), rows 64-127 carry ones x xsq/64 (row term), so PSUM holds the
    complete d2. All PE rows stay active, keeping the HAM clock unthrottled.
  - Columns [0:2048] (two [128,1024] PSUM tiles): ScalarE does out = sqrt(d2)
    straight out of PSUM into f16 (one ACTIVATE per tile; no VectorE work).
  - Columns [2048:2500] (452-col PSUM tile): VectorE evaluates a quadratic
    minimax fit p(y) = c2*(y-r1)*(y-r2) of sqrt in 3 scalar_tensor_tensor
    ops (the tail slice has no fold; xsq rides per-partition AP scalars and
    wsq a broadcast f16 tensor). Fit range sampled host-side; coefficients
    are runtime tensors.
  - Output f16 in a pair-block layout (two m-tiles per [128, 5000] SBUF tile
    -> 10KB-contiguous HBM descriptors), stores split by partition halves
    across the SP HWDGE queue and the Pool SWDGE queue. Host de-permutes and
    upcasts to f32. Max rel err of the whole chain ~6e-3.
"""

import json
import os

import numpy as np

N_CORES = 8
BATCH = 32768
BS = BATCH // N_CORES  # 4096 rows per core
N = 2500
D = 256
M_TILE = 128
M_TILES = BS // M_TILE  # 32

ACT_COLS = 2048  # fold + ScalarE sqrt columns (two 1024-col PSUM tiles)
QW = N - ACT_COLS  # 452 VectorE-quad columns
MM_SLICES = [(0, 512), (512, 512), (1024, 512), (1536, 512), (2048, QW)]

DEFAULT_CFG = {
    "warm_mm": 12,    # PE warm-up matmuls bridging the input-load phase
    "x_chunks": 8,
    "store_swdge": True,  # split stores across SP HWDGE and Pool SWDGE
}

_CACHE = {}


def _cfg():
    cfg = dict(DEFAULT_CFG)
    env = os.environ.get("BASS_SOM_CFG")
    if env:
        cfg.update(json.loads(env))
    return cfg


def _build_bass(cfg=None):
    import concourse.mybir as mybir
    from concourse import bacc
    from concourse.tile import TileContext

    cfg = cfg or _cfg()

    f32 = mybir.dt.float32
    f16 = mybir.dt.float16
    bf16 = mybir.dt.bfloat16
    fp8 = mybir.dt.float8e4
    DR = mybir.MatmulPerfMode.DoubleRow
    ADD = mybir.AluOpType.add
    MULT = mybir.AluOpType.mult
    SQRT = mybir.ActivationFunctionType.Sqrt

    x_chunks = cfg["x_chunks"]
    mc = BS // x_chunks  # m columns per x chunk

    nc = bacc.Bacc("TRN2", target_bir_lowering=False, debug=False)
    xt8_d = nc.dram_tensor("xt8", [128, 2, BS], fp8, kind="ExternalInput")
    wt8_d = nc.dram_tensor("wt8", [128, 2, N], fp8, kind="ExternalInput")
    # fold operands (f16): only the data halves come from HBM, the all-ones
    # halves are memset on device. wf rows 0-63 = wsq/64 (rows 64-127 ones);
    # xf rows 64-127 = xsq/64 (rows 0-63 ones).
    wf_d = nc.dram_tensor("wf", [64, ACT_COLS], f16, kind="ExternalInput")
    xf_d = nc.dram_tensor("xf", [64, BS], f16, kind="ExternalInput")
    wbc_d = nc.dram_tensor("wbc", [128, QW], f16, kind="ExternalInput")
    xr1_d = nc.dram_tensor("xr1", [M_TILE, M_TILES], f32, kind="ExternalInput")
    xr2_d = nc.dram_tensor("xr2", [M_TILE, M_TILES], f32, kind="ExternalInput")
    qc2_d = nc.dram_tensor("qc2", [M_TILE, 1], f32, kind="ExternalInput")
    # Output in pair-block layout: [pair, partition, 2 rows x N]. Each SBUF
    # partition's two rows land contiguously in HBM (10KB descriptors);
    # host de-permutes.
    out = nc.dram_tensor(
        "out", [M_TILES // 2, M_TILE, 2 * N], f16, kind="ExternalOutput"
    )

    with TileContext(nc) as tc:
        with (
            tc.tile_pool(name="wpool", bufs=1) as wpool,
            tc.tile_pool(name="xpool", bufs=1) as xpool,
            tc.tile_pool(name="bpool", bufs=1) as bpool,
            tc.tile_pool(name="opool", bufs=4) as opool,
            tc.tile_pool(name="qpool", bufs=4) as qpool,
            tc.tile_pool(name="pp", bufs=3, space="PSUM") as pp,
            tc.tile_pool(name="pq", bufs=2, space="PSUM") as pq,
        ):
            # --- PE warm-up: no DMA deps, issues at t=0 while inputs load
            # (HAM un-throttle 1.2 -> 2.4 GHz needs ~3.4us of activity; bridge
            # until the first real matmul so it doesn't re-throttle).
            warm_src = bpool.tile([M_TILE, 512], bf16)
            nc.vector.memset(warm_src, 0.0)
            warm_ps = pp.tile([M_TILE, 1024], f32, name="ps")
            for _ in range(cfg["warm_mm"]):
                nc.tensor.matmul(
                    warm_ps[:, :512], lhsT=warm_src[:, :M_TILE], rhs=warm_src,
                    start=True, stop=True,
                )
            # Preload the sqrt ACT table set during the load phase (the
            # implicit ACT_TABLE_LOAD costs ~2.6us at first use otherwise).
            warm_act = bpool.tile([M_TILE, 1], f32)
            nc.scalar.activation(
                warm_act, warm_src[:, 0:1], SQRT, bias=0.0, scale=1.0
            )

            # --- input loads on the SP queue, ordered so m-tile 0 unblocks
            # earliest: small tensors, first w slice, the fold operand (gates
            # the first fold matmul), then the remaining w slices.
            xr1 = bpool.tile([M_TILE, M_TILES], f32)
            nc.sync.dma_start(xr1, xr1_d[:, :])
            xr2 = bpool.tile([M_TILE, M_TILES], f32)
            nc.sync.dma_start(xr2, xr2_d[:, :])
            qc2 = bpool.tile([M_TILE, 1], f32)
            nc.sync.dma_start(qc2, qc2_d[:, :])
            wbc = bpool.tile([128, QW], f16)
            nc.sync.dma_start(wbc, wbc_d[:, :])
            wt8 = wpool.tile([128, 2, N], fp8)
            g0, gw = MM_SLICES[0]
            nc.sync.dma_start(wt8[:, :, g0 : g0 + gw], wt8_d[:, :, g0 : g0 + gw])
            wf = bpool.tile([128, ACT_COLS], f16)
            nc.sync.dma_start(wf[0:64, :], wf_d[:, :])
            nc.vector.memset(wf[64:128, :], 1.0)
            for g0, gw in MM_SLICES[1:]:
                nc.sync.dma_start(
                    wt8[:, :, g0 : g0 + gw], wt8_d[:, :, g0 : g0 + gw]
                )

            x_sb, xf_sb = [], []
            for ci in range(x_chunks):
                xc = xpool.tile([128, 2, mc], fp8, name=f"x{ci}")
                nc.scalar.dma_start(xc, xt8_d[:, :, ci * mc : (ci + 1) * mc])
                x_sb.append(xc)
                # xf rides the otherwise-idle Pool SWDGE queue so the x
                # loads don't fall behind the main loop.
                xfc = xpool.tile([128, mc], f16, name=f"xf{ci}")
                nc.vector.memset(xfc[0:64, :], 1.0)
                nc.gpsimd.dma_start(
                    xfc[64:128, :], xf_d[:, ci * mc : (ci + 1) * mc]
                )
                xf_sb.append(xfc)

            # --- main loop over batch tiles.
            ot2 = None
            pend = None
            for m in range(M_TILES):
                mo = slice((m * M_TILE) % mc, (m * M_TILE) % mc + M_TILE)
                ci = (m * M_TILE) // mc
                xt, xf = x_sb[ci], xf_sb[ci]
                mb = slice(m, m + 1)
                if m % 2 == 0:
                    ot2 = opool.tile([M_TILE, 2 * N], f16, name="ot")
                ot = ot2[:, :N] if m % 2 == 0 else ot2[:, N:]

                ps = [pp.tile([M_TILE, 1024], f32, name="ps") for _ in range(2)]
                psq = pq.tile([M_TILE, QW], f32, name="psq")
                for g0, gw in MM_SLICES:
                    fold = g0 < ACT_COLS
                    dst = (
                        ps[g0 // 1024][:, g0 % 1024 : g0 % 1024 + gw]
                        if fold
                        else psq
                    )
                    nc.tensor.matmul(
                        dst, lhsT=xt[:, :, mo], rhs=wt8[:, :, g0 : g0 + gw],
                        start=True, stop=not fold, perf_mode=DR,
                    )
                    if fold:
                        # full-rank K=128 f16 fold: += wsq[n] + xsq[m]
                        nc.tensor.matmul(
                            dst, lhsT=xf[:, mo], rhs=wf[:, g0 : g0 + gw],
                            start=False, stop=True,
                        )

                # [0:2048]: pure sqrt straight out of PSUM (d2 is complete).
                for t in range(2):
                    nc.scalar.activation(
                        ot[:, t * 1024 : (t + 1) * 1024], ps[t], SQRT,
                        bias=0.0, scale=1.0,
                    )

                # [2048:2500]: VectorE quadratic p(y)=c2*(y-r1)*(y-r2) with
                # y = psq + xsq + wsq built into each factor.
                q1 = qpool.tile([M_TILE, QW], f32, name="q1")
                nc.vector.scalar_tensor_tensor(
                    q1, psq, xr1[:, mb], wbc, ADD, ADD
                )
                q2 = qpool.tile([M_TILE, QW], f32, name="q2")
                nc.vector.scalar_tensor_tensor(
                    q2, psq, xr2[:, mb], wbc, ADD, ADD
                )
                nc.vector.scalar_tensor_tensor(
                    ot[:, ACT_COLS:], q1, qc2[:, 0:1], q2, MULT, MULT
                )

                # Store once per pair, split by partition halves over three
                # queues: SP HWDGE (Sync), Pool SWDGE (GpSimd), and ACT HWDGE
                # (Scalar). Each tops out ~130 GB/s and two can't match the
                # loop's ~280 GB/s output. Scalar's doorbell would stall its
                # ACTIVATE pipeline waiting on tile readiness, so its share
                # is issued one pair LATE (data long since ready).
                if m % 2 == 1:
                    p = m // 2
                    if pend is not None:
                        nc.scalar.dma_start(*pend)
                        pend = None
                    halves = (
                        (out[p, 0:64, :], ot2[0:64, :]),
                        (out[p, 64:128, :], ot2[64:128, :]),
                    )
                    for h, (dst, src) in enumerate(halves):
                        q = (2 * p + h) % 3
                        if q == 2 and cfg["store_swdge"]:
                            pend = (dst, src)
                        elif q == 1 and cfg["store_swdge"]:
                            nc.gpsimd.dma_start(dst, src)
                        else:
                            nc.sync.dma_start(dst, src)
            if pend is not None:
                nc.scalar.dma_start(*pend)

    nc.finalize()
    return nc


def _quad_fit(x, w, xsq, wsq):
    """Sampled-range quadratic minimax fit of sqrt on the d2 range.

    Returns (c2, r1, r2) with sqrt(y) ~= c2*(y-r1)*(y-r2) on the range."""
    rng = np.random.default_rng(12345)
    rows = rng.choice(x.shape[0], 768, replace=False)
    cross = x[rows].astype(np.float32) @ (-2.0 * w.astype(np.float32)).T
    d2 = cross + wsq[None, :].astype(np.float32) + xsq[rows, None].astype(
        np.float32
    )
    smin, smax = float(d2.min()), float(d2.max())
    span = smax - smin
    lo, hi = max(smin - 0.12 * span, 1e-3), smax + 0.12 * span
    yy = np.polynomial.chebyshev.chebpts1(512) * (hi - lo) / 2 + (lo + hi) / 2
    cf = np.polyfit(yy, np.sqrt(yy), 2, w=1.0 / np.sqrt(yy))
    roots = np.roots(cf)
    assert np.isreal(roots).all(), (cf, roots)
    r1, r2 = sorted(roots.real)
    return float(cf[0]), float(r1), float(r2)


def _split64(vals):
    """64 f16 rows summing to vals: 63 equal rows + one residual row."""
    h = (vals / 64.0).astype(np.float16)
    resid = (vals - 63.0 * h.astype(np.float32)).astype(np.float16)
    rows = np.tile(h, (64, 1))
    rows[63] = resid
    return rows  # [64, len(vals)]


def _prep_inputs(x, weights):
    import ml_dtypes

    x = np.ascontiguousarray(np.asarray(x, dtype=np.float32))
    w = np.ascontiguousarray(np.asarray(weights, dtype=np.float32))
    assert x.shape == (BATCH, D), x.shape
    assert w.shape == (N, D), w.shape

    xsq = np.einsum("bd,bd->b", x, x)
    wsq = np.einsum("nd,nd->n", w, w)
    c2, r1, r2 = _quad_fit(x, w, xsq, wsq)

    fp8 = ml_dtypes.float8_e4m3
    xq = x.astype(fp8)  # [B, 256]
    wq = (-2.0 * w).astype(fp8)  # [N, 256]
    # DoubleRow packing: [p, t, cols] with contraction row = 128*t + p.
    wt8 = np.ascontiguousarray(wq.reshape(N, 2, 128).transpose(2, 1, 0))

    # Fold operands (data halves only; the ones halves are device memsets).
    wf = np.ascontiguousarray(_split64(wsq[:ACT_COLS]))  # [64, ACT_COLS]
    wbc = np.tile(wsq[ACT_COLS:].astype(np.float16), (128, 1))  # [128, QW]

    qc2 = np.full((M_TILE, 1), c2, np.float32)

    in_maps = []
    for c in range(N_CORES):
        bs = slice(c * BS, (c + 1) * BS)
        xt8 = np.ascontiguousarray(
            xq[bs].reshape(BS, 2, 128).transpose(2, 1, 0)
        )  # [128, 2, BS]
        xf = np.ascontiguousarray(_split64(xsq[bs]))  # [64, BS]
        xsq_t = np.ascontiguousarray(
            xsq[bs].reshape(M_TILES, M_TILE).T
        )  # [128, 32]
        in_maps.append(
            {
                "xt8": xt8,
                "wt8": wt8,
                "wf": np.ascontiguousarray(wf),
                "xf": xf,
                "wbc": np.ascontiguousarray(wbc),
                "xr1": np.ascontiguousarray(xsq_t - np.float32(r1)),
                "xr2": np.ascontiguousarray(xsq_t - np.float32(r2)),
                "qc2": qc2,
            }
        )
    return in_maps


def _decode_out(arr):
    """[16, 128, 5000] pair-block layout -> [4096, 2500] row order."""
    return (
        arr.reshape(M_TILES // 2, M_TILE, 2, N)
        .transpose(0, 2, 1, 3)
        .reshape(BS, N)
    )


def run(x, weights, trace=False, nc=None, **kwargs):
    from concourse.bass_utils import run_bass_kernel_spmd

    if nc is None:
        if "nc" not in _CACHE:
            _CACHE["nc"] = _build_bass()
        nc = _CACHE["nc"]
    in_maps = _prep_inputs(x, weights)
    res = run_bass_kernel_spmd(
        nc, in_maps, core_ids=list(range(N_CORES)), trace=trace, **kwargs
    )
    out = np.concatenate(
        [
            _decode_out(res.results[c]["out"]).astype(np.float32)
            for c in range(N_CORES)
        ],
        axis=0,
    )
    return out, res


def _get_runner():
    """Build + jit the SPMD executable once; reuse across kernel() calls."""
    if "runner" in _CACHE:
        return _CACHE["runner"]

    import jax
    import concourse.mybir as mybir
    from concourse import bass2jax
    from jax.sharding import Mesh, PartitionSpec
    from jax.experimental.shard_map import shard_map

    bass2jax.install_neuronx_cc_hook()
    if "nc" not in _CACHE:
        _CACHE["nc"] = _build_bass()
    nc = _CACHE["nc"]

    partition_name = (
        nc.partition_id_tensor.name if nc.partition_id_tensor else None
    )
    in_names, out_names, out_avals, zero_templates = [], [], [], []
    for alloc in nc.m.functions[0].allocations:
        if not isinstance(alloc, mybir.MemoryLocationSet):
            continue
        name = alloc.memorylocations[0].name
        if alloc.kind == "ExternalInput":
            if name != partition_name:
                in_names.append(name)
        elif alloc.kind == "ExternalOutput":
            out_names.append(name)
            shape = tuple(alloc.tensor_shape)
            dtype = mybir.dt.np(alloc.dtype)
            out_avals.append(jax.core.ShapedArray(shape, dtype))
            zero_templates.append((shape, dtype))
    n_params = len(in_names)
    n_outs = len(out_names)
    all_names = in_names + out_names
    if partition_name is not None:
        all_names = all_names + [partition_name]
    donate = tuple(range(n_params, n_params + n_outs))

    def _body(*args):
        operands = list(args)
        if partition_name is not None:
            operands.append(bass2jax.partition_id_tensor())
        outs = bass2jax._bass_exec_p.bind(
            *operands,
            out_avals=tuple(out_avals),
            in_names=tuple(all_names),
            out_names=tuple(out_names),
            lowering_input_output_aliases=(),
            sim_require_finite=True,
            sim_require_nnan=True,
            nc=nc,
        )
        return tuple(outs)

    devices = jax.devices()[:N_CORES]
    mesh = Mesh(np.asarray(devices), ("core",))
    specs = (PartitionSpec("core"),) * (n_params + n_outs)
    sharded = jax.jit(
        shard_map(
            _body, mesh=mesh, in_specs=specs, out_specs=specs[:n_outs],
            check_rep=False,
        ),
        donate_argnums=donate,
        keep_unused=True,
    )

    def runner(in_maps):
        concat_in = [
            np.concatenate([m[name] for m in in_maps], axis=0)
            for name in in_names
        ]
        concat_zeros = [
            np.zeros((N_CORES * s[0], *s[1:]), d) for s, d in zero_templates
        ]
        out_arrs = sharded(*concat_in, *concat_zeros)
        return np.asarray(out_arrs[out_names.index("out")])

    _CACHE["runner"] = runner
    return runner


def kernel(x, weights):
    runner = _get_runner()
    in_maps = _prep_inputs(x, weights)
    out = runner(in_maps)  # [8 * 16, 128, 5000] pair-block layout
    out = out.reshape(N_CORES, M_TILES // 2, M_TILE, 2, N)
    out = out.transpose(0, 1, 3, 2, 4).reshape(BATCH, N)
    return np.ascontiguousarray(out.astype(np.float32))


# revision 37
# speedup vs baseline: 1.2989x; 1.2989x over previous
"""Kohonen SOM distance-matrix kernel for Trainium2 (Bass/Tile).

Computes sqrt(||x||^2 + ||w||^2 - 2 x.w) for x [32768, 256] against a codebook
w [2500, 256] -> out [32768, 2500], data-parallel over 8 NeuronCores (batch
sharded, codebook replicated).

Per core (batch shard of 4096 rows, m-tiles of 128):
  - TensorE: fp8(e4m3) DoubleRow matmul (K=256 in one pass) computes
    cross = -2 x.w into PSUM; a second full-rank K=128 f16 matmul per slice
    accumulates BOTH norms on top: rows 0-63 carry wsq/64 x ones (column
    term), rows 64-127 carry ones x xsq/64 (row term), so PSUM holds the
    complete d2. All PE rows stay active, keeping the HAM clock unthrottled.
  - Columns [0:2048] (two [128,1024] PSUM tiles): ScalarE does out = sqrt(d2)
    straight out of PSUM into f16 (one ACTIVATE per tile; no VectorE work).
  - Columns [2048:2500] (452-col PSUM tile): VectorE evaluates a quadratic
    minimax fit p(y) = c2*(y-r1)*(y-r2) of sqrt in 3 scalar_tensor_tensor
    ops (the tail slice has no fold; xsq rides per-partition AP scalars and
    wsq a broadcast f16 tensor). Fit range sampled host-side; coefficients
    are runtime tensors.
  - Output f16 in a pair-block layout (two m-tiles per [128, 5000] SBUF tile
    -> 10KB-contiguous HBM descriptors), stores split by partition halves
    across the SP HWDGE queue and the Pool SWDGE queue. Host de-permutes and
    upcasts to f32. Max rel err of the whole chain ~6e-3.
"""

import json
import os

import numpy as np

N_CORES = 8
BATCH = 32768
BS = BATCH // N_CORES  # 4096 rows per core
N = 2500
D = 256
M_TILE = 128
M_TILES = BS // M_TILE  # 32

ACT_COLS = 2048  # fold + ScalarE sqrt columns (two 1024-col PSUM tiles)
QW = N - ACT_COLS  # 452 VectorE-quad columns
MM_SLICES = [(0, 512), (512, 512), (1024, 512), (1536, 512), (2048, QW)]

DEFAULT_CFG = {
    "warm_mm": 14,    # PE warm-up matmuls bridging the input-load phase
    "x_chunks": 4,
    "store_swdge": True,  # split stores across SP HWDGE and Pool SWDGE
}

_CACHE = {}


def _cfg():
    cfg = dict(DEFAULT_CFG)
    env = os.environ.get("BASS_SOM_CFG")
    if env:
        cfg.update(json.loads(env))
    return cfg


def _build_bass(cfg=None):
    import concourse.mybir as mybir
    from concourse import bacc
    from concourse.tile import TileContext

    cfg = cfg or _cfg()

    f32 = mybir.dt.float32
    f16 = mybir.dt.float16
    bf16 = mybir.dt.bfloat16
    fp8 = mybir.dt.float8e4
    DR = mybir.MatmulPerfMode.DoubleRow
    ADD = mybir.AluOpType.add
    MULT = mybir.AluOpType.mult
    SQRT = mybir.ActivationFunctionType.Sqrt

    x_chunks = cfg["x_chunks"]
    mc = BS // x_chunks  # m columns per x chunk

    nc = bacc.Bacc("TRN2", target_bir_lowering=False, debug=False)
    xt8_d = nc.dram_tensor("xt8", [128, 2, BS], fp8, kind="ExternalInput")
    wt8_d = nc.dram_tensor("wt8", [128, 2, N], fp8, kind="ExternalInput")
    # fold operands (f16): only the data halves come from HBM, the all-ones
    # halves are memset on device. wf rows 0-63 = wsq/64 (rows 64-127 ones);
    # xf rows 64-127 = xsq/64 (rows 0-63 ones).
    wf_d = nc.dram_tensor("wf", [64, ACT_COLS], f16, kind="ExternalInput")
    xf_d = nc.dram_tensor("xf", [64, BS], f16, kind="ExternalInput")
    wbc_d = nc.dram_tensor("wbc", [128, QW], f16, kind="ExternalInput")
    xr1_d = nc.dram_tensor("xr1", [M_TILE, M_TILES], f32, kind="ExternalInput")
    xr2_d = nc.dram_tensor("xr2", [M_TILE, M_TILES], f32, kind="ExternalInput")
    qc2_d = nc.dram_tensor("qc2", [M_TILE, 1], f32, kind="ExternalInput")
    # Output in pair-block layout: [pair, partition, 2 rows x N]. Each SBUF
    # partition's two rows land contiguously in HBM (10KB descriptors);
    # host de-permutes.
    out = nc.dram_tensor(
        "out", [M_TILES // 2, M_TILE, 2 * N], f16, kind="ExternalOutput"
    )

    with TileContext(nc) as tc:
        with (
            tc.tile_pool(name="wpool", bufs=1) as wpool,
            tc.tile_pool(name="xpool", bufs=1) as xpool,
            tc.tile_pool(name="bpool", bufs=1) as bpool,
            tc.tile_pool(name="opool", bufs=4) as opool,
            tc.tile_pool(name="qpool", bufs=4) as qpool,
            tc.tile_pool(name="pp", bufs=3, space="PSUM") as pp,
            tc.tile_pool(name="pq", bufs=2, space="PSUM") as pq,
        ):
            # --- PE warm-up: no DMA deps, issues at t=0 while inputs load
            # (HAM un-throttle 1.2 -> 2.4 GHz needs ~3.4us of activity; bridge
            # until the first real matmul so it doesn't re-throttle).
            warm_src = bpool.tile([M_TILE, 512], bf16)
            nc.vector.memset(warm_src, 0.0)
            warm_ps = pp.tile([M_TILE, 1024], f32, name="ps")
            for _ in range(cfg["warm_mm"]):
                nc.tensor.matmul(
                    warm_ps[:, :512], lhsT=warm_src[:, :M_TILE], rhs=warm_src,
                    start=True, stop=True,
                )
            # Preload the sqrt ACT table set during the load phase (the
            # implicit ACT_TABLE_LOAD costs ~2.6us at first use otherwise).
            warm_act = bpool.tile([M_TILE, 1], f32)
            nc.scalar.activation(
                warm_act, warm_src[:, 0:1], SQRT, bias=0.0, scale=1.0
            )

            # --- input loads on the SP queue, ordered so m-tile 0 unblocks
            # earliest: small tensors, first w slice, the fold operand (gates
            # the first fold matmul), then the remaining w slices.
            xr1 = bpool.tile([M_TILE, M_TILES], f32)
            nc.sync.dma_start(xr1, xr1_d[:, :])
            xr2 = bpool.tile([M_TILE, M_TILES], f32)
            nc.sync.dma_start(xr2, xr2_d[:, :])
            qc2 = bpool.tile([M_TILE, 1], f32)
            nc.sync.dma_start(qc2, qc2_d[:, :])
            wbc = bpool.tile([128, QW], f16)
            nc.sync.dma_start(wbc, wbc_d[:, :])
            wt8 = wpool.tile([128, 2, N], fp8)
            g0, gw = MM_SLICES[0]
            nc.sync.dma_start(wt8[:, :, g0 : g0 + gw], wt8_d[:, :, g0 : g0 + gw])
            wf = bpool.tile([128, ACT_COLS], f16)
            nc.sync.dma_start(wf[0:64, :], wf_d[:, :])
            nc.vector.memset(wf[64:128, :], 1.0)
            for g0, gw in MM_SLICES[1:]:
                nc.sync.dma_start(
                    wt8[:, :, g0 : g0 + gw], wt8_d[:, :, g0 : g0 + gw]
                )

            x_sb, xf_sb = [], []
            for ci in range(x_chunks):
                xc = xpool.tile([128, 2, mc], fp8, name=f"x{ci}")
                nc.scalar.dma_start(xc, xt8_d[:, :, ci * mc : (ci + 1) * mc])
                x_sb.append(xc)
                # xf rides the otherwise-idle Pool SWDGE queue so the x
                # loads don't fall behind the main loop.
                xfc = xpool.tile([128, mc], f16, name=f"xf{ci}")
                nc.vector.memset(xfc[0:64, :], 1.0)
                nc.gpsimd.dma_start(
                    xfc[64:128, :], xf_d[:, ci * mc : (ci + 1) * mc]
                )
                xf_sb.append(xfc)

            # --- main loop over batch tiles.
            ot2 = None
            pend = None
            for m in range(M_TILES):
                mo = slice((m * M_TILE) % mc, (m * M_TILE) % mc + M_TILE)
                ci = (m * M_TILE) // mc
                xt, xf = x_sb[ci], xf_sb[ci]
                mb = slice(m, m + 1)
                if m % 2 == 0:
                    ot2 = opool.tile([M_TILE, 2 * N], f16, name="ot")
                ot = ot2[:, :N] if m % 2 == 0 else ot2[:, N:]

                ps = [pp.tile([M_TILE, 1024], f32, name="ps") for _ in range(2)]
                psq = pq.tile([M_TILE, QW], f32, name="psq")
                for g0, gw in MM_SLICES:
                    fold = g0 < ACT_COLS
                    dst = (
                        ps[g0 // 1024][:, g0 % 1024 : g0 % 1024 + gw]
                        if fold
                        else psq
                    )
                    nc.tensor.matmul(
                        dst, lhsT=xt[:, :, mo], rhs=wt8[:, :, g0 : g0 + gw],
                        start=True, stop=not fold, perf_mode=DR,
                    )
                    if fold:
                        # full-rank K=128 f16 fold: += wsq[n] + xsq[m]
                        nc.tensor.matmul(
                            dst, lhsT=xf[:, mo], rhs=wf[:, g0 : g0 + gw],
                            start=False, stop=True,
                        )

                # [0:2048]: pure sqrt straight out of PSUM (d2 is complete).
                for t in range(2):
                    nc.scalar.activation(
                        ot[:, t * 1024 : (t + 1) * 1024], ps[t], SQRT,
                        bias=0.0, scale=1.0,
                    )

                # [2048:2500]: VectorE quadratic p(y)=c2*(y-r1)*(y-r2) with
                # y = psq + xsq + wsq built into each factor.
                q1 = qpool.tile([M_TILE, QW], f32, name="q1")
                nc.vector.scalar_tensor_tensor(
                    q1, psq, xr1[:, mb], wbc, ADD, ADD
                )
                q2 = qpool.tile([M_TILE, QW], f32, name="q2")
                nc.vector.scalar_tensor_tensor(
                    q2, psq, xr2[:, mb], wbc, ADD, ADD
                )
                nc.vector.scalar_tensor_tensor(
                    ot[:, ACT_COLS:], q1, qc2[:, 0:1], q2, MULT, MULT
                )

                # Store once per pair, split by partition halves over three
                # queues: SP HWDGE (Sync), Pool SWDGE (GpSimd), and ACT HWDGE
                # (Scalar). Each tops out ~130 GB/s and two can't match the
                # loop's ~280 GB/s output. Scalar's doorbell would stall its
                # ACTIVATE pipeline waiting on tile readiness, so its share
                # is issued one pair LATE (data long since ready).
                if m % 2 == 1:
                    p = m // 2
                    nc.sync.dma_start(out[p, 0:64, :], ot2[0:64, :])
                    if cfg["store_swdge"]:
                        nc.gpsimd.dma_start(out[p, 64:128, :], ot2[64:128, :])
                    else:
                        nc.sync.dma_start(out[p, 64:128, :], ot2[64:128, :])

    nc.finalize()
    return nc


def _quad_fit(x, w, xsq, wsq):
    """Sampled-range quadratic minimax fit of sqrt on the d2 range.

    Returns (c2, r1, r2) with sqrt(y) ~= c2*(y-r1)*(y-r2) on the range."""
    rng = np.random.default_rng(12345)
    rows = rng.choice(x.shape[0], 768, replace=False)
    cross = x[rows].astype(np.float32) @ (-2.0 * w.astype(np.float32)).T
    d2 = cross + wsq[None, :].astype(np.float32) + xsq[rows, None].astype(
        np.float32
    )
    smin, smax = float(d2.min()), float(d2.max())
    span = smax - smin
    lo, hi = max(smin - 0.12 * span, 1e-3), smax + 0.12 * span
    yy = np.polynomial.chebyshev.chebpts1(512) * (hi - lo) / 2 + (lo + hi) / 2
    cf = np.polyfit(yy, np.sqrt(yy), 2, w=1.0 / np.sqrt(yy))
    roots = np.roots(cf)
    assert np.isreal(roots).all(), (cf, roots)
    r1, r2 = sorted(roots.real)
    return float(cf[0]), float(r1), float(r2)


def _split64(vals):
    """64 f16 rows summing to vals: 63 equal rows + one residual row."""
    h = (vals / 64.0).astype(np.float16)
    resid = (vals - 63.0 * h.astype(np.float32)).astype(np.float16)
    rows = np.tile(h, (64, 1))
    rows[63] = resid
    return rows  # [64, len(vals)]


def _prep_inputs(x, weights):
    import ml_dtypes

    x = np.ascontiguousarray(np.asarray(x, dtype=np.float32))
    w = np.ascontiguousarray(np.asarray(weights, dtype=np.float32))
    assert x.shape == (BATCH, D), x.shape
    assert w.shape == (N, D), w.shape

    xsq = np.einsum("bd,bd->b", x, x)
    wsq = np.einsum("nd,nd->n", w, w)
    c2, r1, r2 = _quad_fit(x, w, xsq, wsq)

    fp8 = ml_dtypes.float8_e4m3
    xq = x.astype(fp8)  # [B, 256]
    wq = (-2.0 * w).astype(fp8)  # [N, 256]
    # DoubleRow packing: [p, t, cols] with contraction row = 128*t + p.
    wt8 = np.ascontiguousarray(wq.reshape(N, 2, 128).transpose(2, 1, 0))

    # Fold operands (data halves only; the ones halves are device memsets).
    wf = np.ascontiguousarray(_split64(wsq[:ACT_COLS]))  # [64, ACT_COLS]
    wbc = np.tile(wsq[ACT_COLS:].astype(np.float16), (128, 1))  # [128, QW]

    qc2 = np.full((M_TILE, 1), c2, np.float32)

    in_maps = []
    for c in range(N_CORES):
        bs = slice(c * BS, (c + 1) * BS)
        xt8 = np.ascontiguousarray(
            xq[bs].reshape(BS, 2, 128).transpose(2, 1, 0)
        )  # [128, 2, BS]
        xf = np.ascontiguousarray(_split64(xsq[bs]))  # [64, BS]
        xsq_t = np.ascontiguousarray(
            xsq[bs].reshape(M_TILES, M_TILE).T
        )  # [128, 32]
        in_maps.append(
            {
                "xt8": xt8,
                "wt8": wt8,
                "wf": np.ascontiguousarray(wf),
                "xf": xf,
                "wbc": np.ascontiguousarray(wbc),
                "xr1": np.ascontiguousarray(xsq_t - np.float32(r1)),
                "xr2": np.ascontiguousarray(xsq_t - np.float32(r2)),
                "qc2": qc2,
            }
        )
    return in_maps


def _decode_out(arr):
    """[16, 128, 5000] pair-block layout -> [4096, 2500] row order."""
    return (
        arr.reshape(M_TILES // 2, M_TILE, 2, N)
        .transpose(0, 2, 1, 3)
        .reshape(BS, N)
    )


def run(x, weights, trace=False, nc=None, **kwargs):
    from concourse.bass_utils import run_bass_kernel_spmd

    if nc is None:
        if "nc" not in _CACHE:
            _CACHE["nc"] = _build_bass()
        nc = _CACHE["nc"]
    in_maps = _prep_inputs(x, weights)
    res = run_bass_kernel_spmd(
        nc, in_maps, core_ids=list(range(N_CORES)), trace=trace, **kwargs
    )
    out = np.concatenate(
        [
            _decode_out(res.results[c]["out"]).astype(np.float32)
            for c in range(N_CORES)
        ],
        axis=0,
    )
    return out, res


def _get_runner():
    """Build + jit the SPMD executable once; reuse across kernel() calls."""
    if "runner" in _CACHE:
        return _CACHE["runner"]

    import jax
    import concourse.mybir as mybir
    from concourse import bass2jax
    from jax.sharding import Mesh, PartitionSpec
    from jax.experimental.shard_map import shard_map

    bass2jax.install_neuronx_cc_hook()
    if "nc" not in _CACHE:
        _CACHE["nc"] = _build_bass()
    nc = _CACHE["nc"]

    partition_name = (
        nc.partition_id_tensor.name if nc.partition_id_tensor else None
    )
    in_names, out_names, out_avals, zero_templates = [], [], [], []
    for alloc in nc.m.functions[0].allocations:
        if not isinstance(alloc, mybir.MemoryLocationSet):
            continue
        name = alloc.memorylocations[0].name
        if alloc.kind == "ExternalInput":
            if name != partition_name:
                in_names.append(name)
        elif alloc.kind == "ExternalOutput":
            out_names.append(name)
            shape = tuple(alloc.tensor_shape)
            dtype = mybir.dt.np(alloc.dtype)
            out_avals.append(jax.core.ShapedArray(shape, dtype))
            zero_templates.append((shape, dtype))
    n_params = len(in_names)
    n_outs = len(out_names)
    all_names = in_names + out_names
    if partition_name is not None:
        all_names = all_names + [partition_name]
    donate = tuple(range(n_params, n_params + n_outs))

    def _body(*args):
        operands = list(args)
        if partition_name is not None:
            operands.append(bass2jax.partition_id_tensor())
        outs = bass2jax._bass_exec_p.bind(
            *operands,
            out_avals=tuple(out_avals),
            in_names=tuple(all_names),
            out_names=tuple(out_names),
            lowering_input_output_aliases=(),
            sim_require_finite=True,
            sim_require_nnan=True,
            nc=nc,
        )
        return tuple(outs)

    devices = jax.devices()[:N_CORES]
    mesh = Mesh(np.asarray(devices), ("core",))
    specs = (PartitionSpec("core"),) * (n_params + n_outs)
    sharded = jax.jit(
        shard_map(
            _body, mesh=mesh, in_specs=specs, out_specs=specs[:n_outs],
            check_rep=False,
        ),
        donate_argnums=donate,
        keep_unused=True,
    )

    def runner(in_maps):
        concat_in = [
            np.concatenate([m[name] for m in in_maps], axis=0)
            for name in in_names
        ]
        concat_zeros = [
            np.zeros((N_CORES * s[0], *s[1:]), d) for s, d in zero_templates
        ]
        out_arrs = sharded(*concat_in, *concat_zeros)
        return np.asarray(out_arrs[out_names.index("out")])

    _CACHE["runner"] = runner
    return runner


def kernel(x, weights):
    runner = _get_runner()
    in_maps = _prep_inputs(x, weights)
    out = runner(in_maps)  # [8 * 16, 128, 5000] pair-block layout
    out = out.reshape(N_CORES, M_TILES // 2, M_TILE, 2, N)
    out = out.transpose(0, 1, 3, 2, 4).reshape(BATCH, N)
    return np.ascontiguousarray(out.astype(np.float32))
